# revision 15
# baseline (speedup 1.0000x reference)
"""BAD-descriptor kernel for Trainium2 (8 NeuronCores, SPMD over pairs).

Math: the reference gathers from an integral image at
  cy = clip(h + off_y, 0, H-1).astype(int) + r,  y0/y1 = cy -/+ rad(+1)
Because h is an integer grid, clip(h+off).astype(int) == clip(h + floor(off), 0, H-1),
so each box-mean term is the radius-d box-mean image sampled at a clamped
integer 2D shift.  With only 3 radii we precompute, per batch b and d in {1,2,3},
the box-mean image BM_d (edge-replicate semantics of the reference integral
image), padded by 16 with edge replication into BMP_d [256,256]:

  out[b,p] = BMP_{d_p}[b][sy1:sy1+224, sx1:sx1+224]
           - BMP_{d_p}[b][sy2:sy2+224, sx2:sx2+224] - thr_p,
  sy = floor(off_y)+16 in [0,32], sx likewise.

v4 (PE-gather, 112-row blocks, bf16): the 2D-shifted window read runs on the
TENSOR engine; the only HBM traffic is the input image and the bf16 output.

  out[m, n] = sum_k E[k, m] * P[k, sx + n]    E[k, m] = d(k == m + t)

where P is one of six 128-row BMP tiles at starts A = {0,16,32,112,128,144},
picked by sy: block0 (rows 0..111) uses a0 = 16*floor(sy/16), block1 (rows
112..223) uses a1 = a0 + 112, and both share t = sy mod 16.  The tile index
folds into the rhs free-dim dynamic offset (values_load regs, batched 8 pairs
per ~1us TENSOR_LOAD); the per-window lhsT slice E[128+t : 240+t] is staged
by one DVE copy (ldweights cannot take register offsets).  W1 - W2 is free
via PSUM accumulation of +E / -E windows: per pair 4 matmuls (K=128, M=112,
N=448) -> ps0/ps1 [112,448]; ACT/DVE evict with bias=-thr into bf16 staging;
output DMA per 4 pairs on alternating queues.  Plane tiles 0/4 are written
directly by stage-B evictions; tiles 1,2,3,5 are partition-shifted S->S DMA
copies.  Stage-A index arithmetic runs on the idle GpSimd queue; warm-up
matmuls keep the PE DVFS ramp alive between stage B and the gather stream.
"""

import sys

sys.path.insert(0, "/opt/trn_rl_repo")

import numpy as np
import ml_dtypes

import concourse.bass as bass
import concourse.bacc as bacc
import concourse.mybir as mybir
import concourse.tile as tile
from concourse.bass_utils import run_bass_kernel_spmd

BF16_NP = ml_dtypes.bfloat16

B = 2
H = W = 224
P_TOTAL = 256
N_CORES = 8
P_CORE = P_TOTAL // N_CORES  # 32
PAD = 16
RMAX = 3
HP = H + 2 * PAD  # 256 padded image rows/cols
F32 = mybir.dt.float32
BF16 = mybir.dt.bfloat16
I32 = mybir.dt.int32

NB = B * W        # 448 matmul N (b, w)
HB = 112          # output row block height (M)
LGRP = 8          # pairs per register-load batch
OGRP = 4          # pairs per output DMA
EW = 384          # identity block width (j dim) per sign
NT = 6            # plane row-tiles, starts {0,16,32,112,128,144}
TFREE = 3 * B * HP  # 1536 free elems per plane tile
D1 = 3 * TFREE      # block1 rhs offset delta (tiles 3..5 vs 0..2)
N_WARM = 10         # PE warm-up matmuls between stage B and stage C


def _band_matrices() -> np.ndarray:
    """Vertical band matrices with the +-16 replicate pad baked in.

    sdt[0][r, d-1, m]: hs-tile0 row r (x rows 0..127) -> BMP row m
        (m in [0,128): h = max(m-16, 0)).
    sdt[1][k, d-1, m]: hs-tile1 row 96+k -> BMP row 128+m
        (h = min(112+m, 223)).
    entry = #{i in [-d,d] : clip(h+i, 0, H-1) == row}.
    """
    sdt = np.zeros((2, 128, 3, 128), BF16_NP)
    for d in (1, 2, 3):
        for m in range(128):
            h_lo = max(m - PAD, 0)
            h_hi = min(112 + m, H - 1)
            for i in range(-d, d + 1):
                r = min(max(h_lo + i, 0), H - 1)
                if r < 128:
                    sdt[0][r, d - 1, m] += BF16_NP(1.0)
                r = min(max(h_hi + i, 0), H - 1)
                if 96 <= r:
                    sdt[1][r - 96, d - 1, m] += BF16_NP(1.0)
    return sdt


def _shift_identity() -> np.ndarray:
    """e2 [128, 2*EW]: e2[k, j] = d(k == j-128), e2[k, EW+j] = -d(k == j-128)."""
    e = np.zeros((128, 2 * EW), BF16_NP)
    for k in range(128):
        e[k, 128 + k] = 1.0
        e[k, EW + 128 + k] = -1.0
    return e


def build_device_program(nc: bacc.Bacc):
    x_ap = nc.dram_tensor("x", [B, H, W], F32, kind="ExternalInput").ap()
    nthr_ap = nc.dram_tensor("nthr", [1, P_CORE], F32, kind="ExternalInput").ap()
    # per-pair index tables (host-computed): [1, n_lgrp, LGRP, 2] each
    tabpe_ap = nc.dram_tensor("tabpe", [1, P_CORE // LGRP, LGRP, 2], I32,
                              kind="ExternalInput").ap()
    tabdve_ap = nc.dram_tensor("tabdve", [1, P_CORE // LGRP, LGRP, 2], I32,
                               kind="ExternalInput").ap()
    sdt_ap = nc.dram_tensor("sdt", [2, 128, 3, 128], BF16, kind="ExternalInput").ap()
    e2_ap = nc.dram_tensor("e2", [128, 2 * EW], BF16, kind="ExternalInput").ap()
    # bf16 outputs: block0 rows 0..111, block1 rows 112..223; [hpart, p, b, w]
    out0_ap = nc.dram_tensor("out0", [HB, P_CORE, B, W], BF16,
                             kind="ExternalOutput").ap()
    out1_ap = nc.dram_tensor("out1", [HB, P_CORE, B, W], BF16,
                             kind="ExternalOutput").ap()

    with tile.TileContext(nc) as tc:
        build_kernel(tc, out0_ap, out1_ap, x_ap, nthr_ap, tabpe_ap, tabdve_ap,
                     sdt_ap, e2_ap)
    return nc


def build_kernel(tc, out0_ap, out1_ap, x_ap, nthr_ap, tabpe_ap, tabdve_ap,
                 sdt_ap, e2_ap):
    nc = tc.nc
    EngT = mybir.EngineType
    Alu = mybir.AluOpType
    Act = mybir.ActivationFunctionType

    from contextlib import ExitStack
    ctx = ExitStack()
    const_pool = ctx.enter_context(tc.tile_pool(name="const", bufs=1))
    work_pool = ctx.enter_context(tc.tile_pool(name="work", bufs=1))
    psumB_pool = ctx.enter_context(tc.tile_pool(name="psumB", bufs=1, space="PSUM"))
    psum_pool = ctx.enter_context(tc.tile_pool(name="psum", bufs=3, space="PSUM"))
    stage_pool = ctx.enter_context(tc.tile_pool(name="stage", bufs=2))
    estage_pool = ctx.enter_context(tc.tile_pool(name="estage", bufs=9))

    # ---------------- input DMAs (x first: it gates the whole chain) -----
    part_rows = ((0, 128), (96, 128))  # (row0, nrows) x-row tiles (overlapping)
    xts = []
    for j, (r0, nr) in enumerate(part_rows):
        xt = work_pool.tile([nr, B, W + 2 * RMAX], F32, tag=f"xt_{j}")
        for b in range(B):
            eng = nc.sync if b == 0 else nc.scalar
            eng.dma_start(out=xt[:, b, RMAX:RMAX + W], in_=x_ap[b, r0:r0 + nr, :])
        xts.append(xt)
    sdt_lo = const_pool.tile([128, 3, 128], BF16, tag="sdt_lo")
    sdt_hi = const_pool.tile([128, 3, 128], BF16, tag="sdt_hi")
    nc.sync.dma_start(out=sdt_lo[:], in_=sdt_ap[0])
    nc.scalar.dma_start(out=sdt_hi[:], in_=sdt_ap[1])
    n_lgrp = P_CORE // LGRP
    tab_pe = const_pool.tile([1, n_lgrp, LGRP, 2], I32, tag="tab_pe")
    tab_dve = const_pool.tile([1, n_lgrp, LGRP, 2], I32, tag="tab_dve")
    nc.scalar.dma_start(out=tab_pe[:], in_=tabpe_ap[:])
    nc.scalar.dma_start(out=tab_dve[:], in_=tabdve_ap[:])
    e2t = const_pool.tile([128, 2 * EW], BF16, tag="e2t")
    nc.sync.dma_start(out=e2t[:], in_=e2_ap[:])
    negthr = const_pool.tile([128, P_CORE], F32, tag="negthr")
    nc.scalar.dma_start(out=negthr[:],
                        in_=nthr_ap[0:1, :].to_broadcast((128, P_CORE)))

    # ------- DVE: x cast + pads + horizontal taps (j0 fully first) -------
    hs = {1: [], 2: [], 3: []}
    for j, (r0, nr) in enumerate(part_rows):
        xt = xts[j]
        xh = work_pool.tile([nr, B, W + 2 * RMAX], BF16, tag=f"xh_{j}")
        nc.vector.tensor_copy(out=xh[:, :, RMAX:RMAX + W],
                              in_=xt[:, :, RMAX:RMAX + W])
        nc.vector.tensor_copy(
            out=xh[:, :, 0:RMAX],
            in_=xh[:, :, RMAX:RMAX + 1].to_broadcast((nr, B, RMAX)))
        nc.vector.tensor_copy(
            out=xh[:, :, RMAX + W:],
            in_=xh[:, :, RMAX + W - 1:RMAX + W].to_broadcast((nr, B, RMAX)))
        eng = nc.vector
        h1 = work_pool.tile([nr, B, W], BF16, tag=f"hs1_{j}")
        h2 = work_pool.tile([nr, B, W], BF16, tag=f"hs2_{j}")
        h3 = work_pool.tile([nr, B, W], BF16, tag=f"hs3_{j}")
        ta = work_pool.tile([nr, B, W], BF16, tag=f"hta_{j}")
        sl = lambda c: xh[:, :, c:c + W]
        eng.tensor_tensor(out=ta[:], in0=sl(2), in1=sl(3), op=Alu.add)
        eng.tensor_tensor(out=h1[:], in0=ta[:], in1=sl(4), op=Alu.add)
        eng.tensor_tensor(out=ta[:], in0=sl(1), in1=sl(5), op=Alu.add)
        eng.tensor_tensor(out=h2[:], in0=h1[:], in1=ta[:], op=Alu.add)
        eng.tensor_tensor(out=ta[:], in0=sl(0), in1=sl(6), op=Alu.add)
        eng.tensor_tensor(out=h3[:], in0=h2[:], in1=ta[:], op=Alu.add)
        hs[1].append(h1)
        hs[2].append(h2)
        hs[3].append(h3)

    # ---------------- Stage B: plane tiles (PE matmul + ACT evict) -------
    # plane tensor [128, NT, 3, B, HP] bf16; tile starts {0,16,32,112,128,144}.
    pl = work_pool.tile([128, NT, 3, B, HP], BF16, tag="pl")
    for j, tix in ((0, 0), (1, 4)):
        for d in (1, 2, 3):
            area = float((2 * d + 1) ** 2)
            ps = psumB_pool.tile([128, NB], F32, tag=f"bps{j}")
            sdt_t = sdt_lo if j == 0 else sdt_hi
            nc.tensor.matmul(out=ps[:], lhsT=sdt_t[:, d - 1, :],
                             rhs=hs[d][j][:].rearrange("r b w -> r (b w)"),
                             start=True, stop=True)
            nc.scalar.activation(pl[:, tix, d - 1, :, PAD:PAD + W],
                                 ps[:].rearrange("r (b w) -> r b w", b=B),
                                 Act.Copy, scale=1.0 / area)
            nc.vector.tensor_copy(
                out=pl[:, tix, d - 1, :, 0:PAD],
                in_=pl[:, tix, d - 1, :, PAD:PAD + 1].to_broadcast((128, B, PAD)))
            nc.vector.tensor_copy(
                out=pl[:, tix, d - 1, :, PAD + W:],
                in_=pl[:, tix, d - 1, :, PAD + W - 1:PAD + W].to_broadcast(
                    (128, B, PAD)))

    # warm-up matmuls: keep the PE busy (DVFS ramp) while the S->S plane
    # copies run; results are discarded.
    for wmm in range(N_WARM):
        wps = psumB_pool.tile([128, NB], F32, tag=f"bps{wmm % 2}")
        nc.tensor.matmul(out=wps[:], lhsT=sdt_lo[:, 0, :], rhs=e2t[:, 0:NB],
                         start=True, stop=True)

    # S->S partition-shifted copies (all d at once):
    # (dst_tile, src_tile, src_p0, n, dst_p0); tile5 rows 112.. are filler
    # (finite values only, never selected by E).
    for k, (dt_, st_, sp, n, dp) in enumerate((
            (1, 0, 16, 112, 0), (1, 4, 0, 16, 112),
            (2, 0, 32, 96, 0), (2, 4, 0, 32, 96),
            (3, 0, 112, 16, 0), (3, 4, 0, 112, 16),
            (5, 4, 16, 112, 0), (5, 4, 112, 16, 112))):
        eng = nc.sync if k % 2 == 0 else nc.scalar
        eng.dma_start(out=pl[dp:dp + n, dt_, :, :, :],
                      in_=pl[sp:sp + n, st_, :, :, :])

    # ---------------- Stage C: PE gather ----------------
    pl_base = pl[:].offset
    e2_base = e2t[:].offset
    assert isinstance(pl_base, int) and isinstance(e2_base, int)
    PL_AP = [[NT * TFREE, 128], [HP, B], [1, W]]   # [128, b, 224] window view
    MAX_RHS = 2 * TFREE + 2 * 512 + 2 * PAD

    def rhs_ap(off, extra):
        return bass.AP(pl[:].tensor, pl_base + extra + off,
                       [r[:] for r in PL_AP])

    def lhs_ap(off):
        return bass.AP(e2t[:].tensor, e2_base + off, [[2 * EW, 128], [1, HB]])

    sts = {}
    for g in range(n_lgrp):
        _, pe_vals = nc.values_load_multi_w_load_instructions(
            tab_pe[0:1, g, :, :], engines=[EngT.PE],
            min_val=0, max_val=MAX_RHS, skip_runtime_bounds_check=True)
        _, dve_vals = nc.values_load_multi_w_load_instructions(
            tab_dve[0:1, g, :, :], engines=[EngT.DVE],
            min_val=0, max_val=EW + 128 + 16, skip_runtime_bounds_check=True)
        # stage all LGRP pairs' E slices up front so the PE never waits
        ess = []
        for i in range(LGRP):
            es1 = estage_pool.tile([128, HB], BF16, tag="es1")
            es2 = estage_pool.tile([128, HB], BF16, tag="es2")
            nc.vector.tensor_copy(out=es1[:], in_=lhs_ap(dve_vals[2 * i]))
            nc.vector.tensor_copy(out=es2[:], in_=lhs_ap(dve_vals[2 * i + 1]))
            ess.append((es1, es2))
        for i in range(LGRP):
            p = g * LGRP + i
            og, oi = p // OGRP, p % OGRP
            if oi == 0:
                st0 = stage_pool.tile([HB, OGRP, B, W], BF16, tag="st0")
                st1 = stage_pool.tile([HB, OGRP, B, W], BF16, tag="st1")
                sts[og] = (st0, st1)
            st0, st1 = sts[og]
            or1, or2 = pe_vals[2 * i], pe_vals[2 * i + 1]
            es1, es2 = ess[i]
            ps0 = psum_pool.tile([HB, NB], F32, tag="ps0")
            ps1 = psum_pool.tile([HB, NB], F32, tag="ps1")
            nc.tensor.matmul(out=ps0[:], lhsT=es1[:], rhs=rhs_ap(or1, 0),
                             start=True, stop=False)
            nc.tensor.matmul(out=ps1[:], lhsT=es1[:], rhs=rhs_ap(or1, D1),
                             start=True, stop=False)
            nc.tensor.matmul(out=ps0[:], lhsT=es2[:], rhs=rhs_ap(or2, 0),
                             start=False, stop=True)
            nc.tensor.matmul(out=ps1[:], lhsT=es2[:], rhs=rhs_ap(or2, D1),
                             start=False, stop=True)
            nc.scalar.activation(st0[:, oi, :, :],
                                 ps0[:].rearrange("r (b w) -> r b w", b=B),
                                 Act.Identity, bias=negthr[0:HB, p:p + 1])
            ps1_on_act = (i % 3 == 2) and g < n_lgrp - 1
            if ps1_on_act:
                nc.scalar.activation(st1[:, oi, :, :],
                                     ps1[:].rearrange("r (b w) -> r b w", b=B),
                                     Act.Identity, bias=negthr[0:HB, p:p + 1])
            else:
                nc.vector.tensor_scalar_add(
                    out=st1[:, oi, :, :],
                    in0=ps1[:].rearrange("r (b w) -> r b w", b=B),
                    scalar1=negthr[0:HB, p:p + 1])
            if oi == OGRP - 1:
                e0 = nc.sync if og % 2 == 0 else nc.scalar
                e1 = nc.scalar if og % 2 == 0 else nc.sync
                e0.dma_start(out=out0_ap[:, og * OGRP:(og + 1) * OGRP],
                             in_=st0[:])
                e1.dma_start(out=out1_ap[:, og * OGRP:(og + 1) * OGRP],
                             in_=st1[:])

    ctx.close()


_COMPILED = {}


def _get_compiled():
    if "nc" not in _COMPILED:
        nc = bacc.Bacc("TRN2", target_bir_lowering=False, debug=False,
                       num_devices=N_CORES)
        build_device_program(nc)
        nc.compile()
        _COMPILED["nc"] = nc
    return _COMPILED["nc"]


def _ensure_ntff_hook():
    """The agent image's antenv lacks axon_hooks; shim it so trace=True can
    drive NTFF profiling via the boot module's ctypes hook (test-only path)."""
    import types

    try:
        from antenv.axon_hooks import get_axon_ntff_profile_hook  # noqa: F401
        return
    except ImportError:
        pass
    import antenv

    mod = types.ModuleType("antenv.axon_hooks")
    _hook = [None]
    mod.set_axon_ntff_profile_hook = lambda h: _hook.__setitem__(0, h)
    mod.get_axon_ntff_profile_hook = lambda: _hook[0]
    sys.modules["antenv.axon_hooks"] = mod
    antenv.axon_hooks = mod
    from trn_agent_boot.trn_boot import _ntff_profile_via_ctypes

    mod.set_axon_ntff_profile_hook(
        _ntff_profile_via_ctypes("/opt/axon/libaxon_pjrt.so"))


def run(inputs: dict, trace: bool = False):
    """Run on the 8 cores. Returns (full output [B,256,H,W], exec_time_ns|None)."""
    x = np.asarray(inputs["x"], dtype=np.float32).reshape(B, H, W)
    offset_x1 = np.asarray(inputs["offset_x1"], np.float32)
    offset_x2 = np.asarray(inputs["offset_x2"], np.float32)
    offset_y1 = np.asarray(inputs["offset_y1"], np.float32)
    offset_y2 = np.asarray(inputs["offset_y2"], np.float32)
    radii = np.asarray(inputs["radii"]).astype(np.int32)
    thresholds = np.asarray(inputs["thresholds"], np.float32)

    sdt = _band_matrices()
    e2 = _shift_identity()
    nc = _get_compiled()

    # host-side index prep: sy/sx = clip(floor(off), -16, 16)+16; b, t from sy
    def sclip(off):
        return np.clip(np.floor(off), -PAD, PAD).astype(np.int64) + PAD

    sy1, sx1 = sclip(offset_y1), sclip(offset_x1)
    sy2, sx2 = sclip(offset_y2), sclip(offset_x2)
    d0 = np.clip(radii.astype(np.int64), 1, 3) - 1
    n_lgrp = P_CORE // LGRP

    def tabs(sy, sx):
        b = (sy >= 16).astype(np.int64) + (sy >= 32).astype(np.int64)
        t = sy - 16 * b
        pe = b * TFREE + d0 * 512 + sx
        return pe, t

    pe1, t1 = tabs(sy1, sx1)
    pe2, t2 = tabs(sy2, sx2)
    dve1 = 128 + t1
    dve2 = EW + 128 + t2
    tab_pe = np.stack([pe1, pe2], axis=-1).astype(np.int32)    # [P_TOTAL, 2]
    tab_dve = np.stack([dve1, dve2], axis=-1).astype(np.int32)

    in_maps = []
    for c in range(N_CORES):
        sl = slice(c * P_CORE, (c + 1) * P_CORE)
        in_maps.append({
            "x": x,
            "nthr": -thresholds[sl].reshape(1, P_CORE).astype(np.float32),
            "tabpe": tab_pe[sl].reshape(1, n_lgrp, LGRP, 2),
            "tabdve": tab_dve[sl].reshape(1, n_lgrp, LGRP, 2),
            "sdt": sdt,
            "e2": e2,
        })

    if trace:
        _ensure_ntff_hook()
    res = run_bass_kernel_spmd(nc, in_maps, list(range(N_CORES)), trace=trace)
    # per-core out0/out1 [112, P_CORE, B, W] bf16 (rows 0..111 / 112..223)
    full = np.empty((B, P_TOTAL, H, W), np.float32)
    for c in range(N_CORES):
        o0 = np.asarray(res.results[c]["out0"]).astype(np.float32)
        o1 = np.asarray(res.results[c]["out1"]).astype(np.float32)
        sl = slice(c * P_CORE, (c + 1) * P_CORE)
        full[:, sl, :HB, :] = o0.transpose(2, 1, 0, 3)
        full[:, sl, HB:, :] = o1.transpose(2, 1, 0, 3)
    return full, res.exec_time_ns


def kernel(x, offset_x1, offset_x2, offset_y1, offset_y2, radii, thresholds,
           max_radius):
    out, _ = run({
        "x": x, "offset_x1": offset_x1, "offset_x2": offset_x2,
        "offset_y1": offset_y1, "offset_y2": offset_y2,
        "radii": radii, "thresholds": thresholds, "max_radius": max_radius,
    })
    return out


if __name__ == "__main__":
    # smoke test with random data
    rng = np.random.default_rng(0)
    out = kernel(
        x=rng.standard_normal((B, 1, H, W), dtype=np.float32),
        offset_x1=rng.uniform(-16, 16, P_TOTAL).astype(np.float32),
        offset_x2=rng.uniform(-16, 16, P_TOTAL).astype(np.float32),
        offset_y1=rng.uniform(-16, 16, P_TOTAL).astype(np.float32),
        offset_y2=rng.uniform(-16, 16, P_TOTAL).astype(np.float32),
        radii=rng.integers(1, 4, P_TOTAL).astype(np.int32),
        thresholds=(rng.standard_normal(P_TOTAL) * 0.1).astype(np.float32),
        max_radius=3,
    )
    print("out", out.shape, out.dtype, float(np.abs(out).max()))


# revision 16
# speedup vs baseline: 1.0160x; 1.0160x over previous
"""BAD-descriptor kernel for Trainium2 (8 NeuronCores, SPMD over pairs).

Math: the reference gathers from an integral image at
  cy = clip(h + off_y, 0, H-1).astype(int) + r,  y0/y1 = cy -/+ rad(+1)
Because h is an integer grid, clip(h+off).astype(int) == clip(h + floor(off), 0, H-1),
so each box-mean term is the radius-d box-mean image sampled at a clamped
integer 2D shift.  With only 3 radii we precompute, per batch b and d in {1,2,3},
the box-mean image BM_d (edge-replicate semantics of the reference integral
image), padded by 16 with edge replication into BMP_d [256,256]:

  out[b,p] = BMP_{d_p}[b][sy1:sy1+224, sx1:sx1+224]
           - BMP_{d_p}[b][sy2:sy2+224, sx2:sx2+224] - thr_p,
  sy = floor(off_y)+16 in [0,32], sx likewise.

v4 (PE-gather, 112-row blocks, bf16): the 2D-shifted window read runs on the
TENSOR engine; the only HBM traffic is the input image and the bf16 output.

  out[m, n] = sum_k E[k, m] * P[k, sx + n]    E[k, m] = d(k == m + t)

where P is one of six 128-row BMP tiles at starts A = {0,16,32,112,128,144},
picked by sy: block0 (rows 0..111) uses a0 = 16*floor(sy/16), block1 (rows
112..223) uses a1 = a0 + 112, and both share t = sy mod 16.  The tile index
folds into the rhs free-dim dynamic offset (values_load regs, batched 8 pairs
per ~1us TENSOR_LOAD); the per-window lhsT slice E[128+t : 240+t] is staged
by one DVE copy (ldweights cannot take register offsets).  W1 - W2 is free
via PSUM accumulation of +E / -E windows: per pair 4 matmuls (K=128, M=112,
N=448) -> ps0/ps1 [112,448]; ACT/DVE evict with bias=-thr into bf16 staging;
output DMA per 4 pairs on alternating queues.  Plane tiles 0/4 are written
directly by stage-B evictions; tiles 1,2,3,5 are partition-shifted S->S DMA
copies.  Stage-A index arithmetic runs on the idle GpSimd queue; warm-up
matmuls keep the PE DVFS ramp alive between stage B and the gather stream.
"""

import sys

sys.path.insert(0, "/opt/trn_rl_repo")

import numpy as np
import ml_dtypes

import concourse.bass as bass
import concourse.bacc as bacc
import concourse.mybir as mybir
import concourse.tile as tile
from concourse.bass_utils import run_bass_kernel_spmd

BF16_NP = ml_dtypes.bfloat16

B = 2
H = W = 224
P_TOTAL = 256
N_CORES = 8
P_CORE = P_TOTAL // N_CORES  # 32
PAD = 16
RMAX = 3
HP = H + 2 * PAD  # 256 padded image rows/cols
F32 = mybir.dt.float32
BF16 = mybir.dt.bfloat16
I32 = mybir.dt.int32

NB = B * W        # 448 matmul N (b, w)
HB = 112          # output row block height (M)
LGRP = 8          # pairs per register-load batch
OGRP = 4          # pairs per output DMA
EW = 384          # identity block width (j dim) per sign
NT = 6            # plane row-tiles, starts {0,16,32,112,128,144}
TFREE = 3 * B * HP  # 1536 free elems per plane tile
D1 = 3 * TFREE      # block1 rhs offset delta (tiles 3..5 vs 0..2)
N_WARM = 10         # PE warm-up matmuls between stage B and stage C


def _band_matrices() -> np.ndarray:
    """Vertical band matrices with the +-16 replicate pad baked in.

    sdt[0][r, d-1, m]: hs-tile0 row r (x rows 0..127) -> BMP row m
        (m in [0,128): h = max(m-16, 0)).
    sdt[1][k, d-1, m]: hs-tile1 row 96+k -> BMP row 128+m
        (h = min(112+m, 223)).
    entry = #{i in [-d,d] : clip(h+i, 0, H-1) == row}.
    """
    sdt = np.zeros((2, 128, 3, 128), BF16_NP)
    for d in (1, 2, 3):
        for m in range(128):
            h_lo = max(m - PAD, 0)
            h_hi = min(112 + m, H - 1)
            for i in range(-d, d + 1):
                r = min(max(h_lo + i, 0), H - 1)
                if r < 128:
                    sdt[0][r, d - 1, m] += BF16_NP(1.0)
                r = min(max(h_hi + i, 0), H - 1)
                if 96 <= r:
                    sdt[1][r - 96, d - 1, m] += BF16_NP(1.0)
    return sdt


def _shift_identity() -> np.ndarray:
    """e2 [128, 2*EW]: e2[k, j] = d(k == j-128), e2[k, EW+j] = -d(k == j-128)."""
    e = np.zeros((128, 2 * EW), BF16_NP)
    for k in range(128):
        e[k, 128 + k] = 1.0
        e[k, EW + 128 + k] = -1.0
    return e


def build_device_program(nc: bacc.Bacc):
    x_ap = nc.dram_tensor("x", [B, H, W], F32, kind="ExternalInput").ap()
    nthr_ap = nc.dram_tensor("nthr", [1, P_CORE], F32, kind="ExternalInput").ap()
    # per-pair index tables (host-computed): [pe/dve, n_lgrp, LGRP, 2]
    tab_ap = nc.dram_tensor("tab", [1, 2, P_CORE // LGRP, LGRP, 2], I32,
                            kind="ExternalInput").ap()
    sdt_ap = nc.dram_tensor("sdt", [2, 128, 3, 128], BF16, kind="ExternalInput").ap()
    e2_ap = nc.dram_tensor("e2", [128, 2 * EW], BF16, kind="ExternalInput").ap()
    # bf16 outputs: block0 rows 0..111, block1 rows 112..223; [hpart, p, b, w]
    out0_ap = nc.dram_tensor("out0", [HB, P_CORE, B, W], BF16,
                             kind="ExternalOutput").ap()
    out1_ap = nc.dram_tensor("out1", [HB, P_CORE, B, W], BF16,
                             kind="ExternalOutput").ap()

    with tile.TileContext(nc) as tc:
        build_kernel(tc, out0_ap, out1_ap, x_ap, nthr_ap, tab_ap, sdt_ap, e2_ap)
    return nc


def build_kernel(tc, out0_ap, out1_ap, x_ap, nthr_ap, tab_ap, sdt_ap, e2_ap):
    nc = tc.nc
    EngT = mybir.EngineType
    Alu = mybir.AluOpType
    Act = mybir.ActivationFunctionType

    from contextlib import ExitStack
    ctx = ExitStack()
    const_pool = ctx.enter_context(tc.tile_pool(name="const", bufs=1))
    work_pool = ctx.enter_context(tc.tile_pool(name="work", bufs=1))
    psumB_pool = ctx.enter_context(tc.tile_pool(name="psumB", bufs=1, space="PSUM"))
    psum_pool = ctx.enter_context(tc.tile_pool(name="psum", bufs=3, space="PSUM"))
    stage_pool = ctx.enter_context(tc.tile_pool(name="stage", bufs=2))
    estage_pool = ctx.enter_context(tc.tile_pool(name="estage", bufs=9))

    # -------- input DMAs (tab first on sync: it unblocks the PE queue; ---
    # -------- then x, which gates the tap chain) -------------------------
    n_lgrp = P_CORE // LGRP
    tab = const_pool.tile([1, 2, n_lgrp, LGRP, 2], I32, tag="tab")
    nc.sync.dma_start(out=tab[:], in_=tab_ap[:])
    part_rows = ((0, 128), (96, 128))  # (row0, nrows) x-row tiles (overlapping)
    xts = []
    for j, (r0, nr) in enumerate(part_rows):
        xt = work_pool.tile([nr, B, W + 2 * RMAX], F32, tag=f"xt_{j}")
        for b in range(B):
            eng = nc.sync if b == 0 else nc.scalar
            eng.dma_start(out=xt[:, b, RMAX:RMAX + W], in_=x_ap[b, r0:r0 + nr, :])
        xts.append(xt)
    sdt_lo = const_pool.tile([128, 3, 128], BF16, tag="sdt_lo")
    sdt_hi = const_pool.tile([128, 3, 128], BF16, tag="sdt_hi")
    nc.sync.dma_start(out=sdt_lo[:], in_=sdt_ap[0])
    nc.scalar.dma_start(out=sdt_hi[:], in_=sdt_ap[1])
    e2t = const_pool.tile([128, 2 * EW], BF16, tag="e2t")
    nc.sync.dma_start(out=e2t[:], in_=e2_ap[:])
    negthr = const_pool.tile([128, P_CORE], F32, tag="negthr")
    nc.scalar.dma_start(out=negthr[:],
                        in_=nthr_ap[0:1, :].to_broadcast((128, P_CORE)))

    # ------- DVE: x cast + pads + horizontal taps (j0 fully first) -------
    hs = {1: [], 2: [], 3: []}
    for j, (r0, nr) in enumerate(part_rows):
        xt = xts[j]
        xh = work_pool.tile([nr, B, W + 2 * RMAX], BF16, tag=f"xh_{j}")
        nc.vector.tensor_copy(out=xh[:, :, RMAX:RMAX + W],
                              in_=xt[:, :, RMAX:RMAX + W])
        nc.vector.tensor_copy(
            out=xh[:, :, 0:RMAX],
            in_=xh[:, :, RMAX:RMAX + 1].to_broadcast((nr, B, RMAX)))
        nc.vector.tensor_copy(
            out=xh[:, :, RMAX + W:],
            in_=xh[:, :, RMAX + W - 1:RMAX + W].to_broadcast((nr, B, RMAX)))
        eng = nc.vector
        h1 = work_pool.tile([nr, B, W], BF16, tag=f"hs1_{j}")
        h2 = work_pool.tile([nr, B, W], BF16, tag=f"hs2_{j}")
        h3 = work_pool.tile([nr, B, W], BF16, tag=f"hs3_{j}")
        ta = work_pool.tile([nr, B, W], BF16, tag=f"hta_{j}")
        sl = lambda c: xh[:, :, c:c + W]
        eng.tensor_tensor(out=ta[:], in0=sl(2), in1=sl(3), op=Alu.add)
        eng.tensor_tensor(out=h1[:], in0=ta[:], in1=sl(4), op=Alu.add)
        eng.tensor_tensor(out=ta[:], in0=sl(1), in1=sl(5), op=Alu.add)
        eng.tensor_tensor(out=h2[:], in0=h1[:], in1=ta[:], op=Alu.add)
        eng.tensor_tensor(out=ta[:], in0=sl(0), in1=sl(6), op=Alu.add)
        eng.tensor_tensor(out=h3[:], in0=h2[:], in1=ta[:], op=Alu.add)
        hs[1].append(h1)
        hs[2].append(h2)
        hs[3].append(h3)

    # ---------------- Stage B: plane tiles (PE matmul + ACT evict) -------
    # plane tensor [128, NT, 3, B, HP] bf16; tile starts {0,16,32,112,128,144}.
    pl = work_pool.tile([128, NT, 3, B, HP], BF16, tag="pl")
    for j, tix in ((0, 0), (1, 4)):
        for d in (1, 2, 3):
            area = float((2 * d + 1) ** 2)
            ps = psumB_pool.tile([128, NB], F32, tag=f"bps{j}")
            sdt_t = sdt_lo if j == 0 else sdt_hi
            nc.tensor.matmul(out=ps[:], lhsT=sdt_t[:, d - 1, :],
                             rhs=hs[d][j][:].rearrange("r b w -> r (b w)"),
                             start=True, stop=True)
            nc.scalar.activation(pl[:, tix, d - 1, :, PAD:PAD + W],
                                 ps[:].rearrange("r (b w) -> r b w", b=B),
                                 Act.Copy, scale=1.0 / area)
            nc.vector.tensor_copy(
                out=pl[:, tix, d - 1, :, 0:PAD],
                in_=pl[:, tix, d - 1, :, PAD:PAD + 1].to_broadcast((128, B, PAD)))
            nc.vector.tensor_copy(
                out=pl[:, tix, d - 1, :, PAD + W:],
                in_=pl[:, tix, d - 1, :, PAD + W - 1:PAD + W].to_broadcast(
                    (128, B, PAD)))

    # warm-up matmuls: keep the PE busy (DVFS ramp) while the S->S plane
    # copies run; results are discarded.
    for wmm in range(N_WARM):
        wps = psumB_pool.tile([128, NB], F32, tag=f"bps{wmm % 2}")
        nc.tensor.matmul(out=wps[:], lhsT=sdt_lo[:, 0, :], rhs=e2t[:, 0:NB],
                         start=True, stop=True)

    # S->S partition-shifted copies (all d at once):
    # (dst_tile, src_tile, src_p0, n, dst_p0); tile5 rows 112.. are filler
    # (finite values only, never selected by E).
    for k, (dt_, st_, sp, n, dp) in enumerate((
            (1, 0, 16, 112, 0), (1, 4, 0, 16, 112),
            (2, 0, 32, 96, 0), (2, 4, 0, 32, 96),
            (3, 0, 112, 16, 0), (3, 4, 0, 112, 16),
            (5, 4, 16, 112, 0), (5, 4, 112, 16, 112))):
        eng = nc.sync if k % 2 == 0 else nc.scalar
        eng.dma_start(out=pl[dp:dp + n, dt_, :, :, :],
                      in_=pl[sp:sp + n, st_, :, :, :])

    # ---------------- Stage C: PE gather ----------------
    pl_base = pl[:].offset
    e2_base = e2t[:].offset
    assert isinstance(pl_base, int) and isinstance(e2_base, int)
    PL_AP = [[NT * TFREE, 128], [HP, B], [1, W]]   # [128, b, 224] window view
    MAX_RHS = 2 * TFREE + 2 * 512 + 2 * PAD

    def rhs_ap(off, extra):
        return bass.AP(pl[:].tensor, pl_base + extra + off,
                       [r[:] for r in PL_AP])

    def lhs_ap(off):
        return bass.AP(e2t[:].tensor, e2_base + off, [[2 * EW, 128], [1, HB]])

    sts = {}
    for g in range(n_lgrp):
        _, pe_vals = nc.values_load_multi_w_load_instructions(
            tab[0:1, 0, g, :, :], engines=[EngT.PE],
            min_val=0, max_val=MAX_RHS, skip_runtime_bounds_check=True)
        _, dve_vals = nc.values_load_multi_w_load_instructions(
            tab[0:1, 1, g, :, :], engines=[EngT.DVE],
            min_val=0, max_val=EW + 128 + 16, skip_runtime_bounds_check=True)
        # stage all LGRP pairs' E slices up front so the PE never waits
        ess = []
        for i in range(LGRP):
            es1 = estage_pool.tile([128, HB], BF16, tag="es1")
            es2 = estage_pool.tile([128, HB], BF16, tag="es2")
            nc.vector.tensor_copy(out=es1[:], in_=lhs_ap(dve_vals[2 * i]))
            nc.vector.tensor_copy(out=es2[:], in_=lhs_ap(dve_vals[2 * i + 1]))
            ess.append((es1, es2))
        for i in range(LGRP):
            p = g * LGRP + i
            og, oi = p // OGRP, p % OGRP
            if oi == 0:
                st0 = stage_pool.tile([HB, OGRP, B, W], BF16, tag="st0")
                st1 = stage_pool.tile([HB, OGRP, B, W], BF16, tag="st1")
                sts[og] = (st0, st1)
            st0, st1 = sts[og]
            or1, or2 = pe_vals[2 * i], pe_vals[2 * i + 1]
            es1, es2 = ess[i]
            ps0 = psum_pool.tile([HB, NB], F32, tag="ps0")
            ps1 = psum_pool.tile([HB, NB], F32, tag="ps1")
            nc.tensor.matmul(out=ps0[:], lhsT=es1[:], rhs=rhs_ap(or1, 0),
                             start=True, stop=False)
            nc.tensor.matmul(out=ps1[:], lhsT=es1[:], rhs=rhs_ap(or1, D1),
                             start=True, stop=False)
            nc.tensor.matmul(out=ps0[:], lhsT=es2[:], rhs=rhs_ap(or2, 0),
                             start=False, stop=True)
            nc.tensor.matmul(out=ps1[:], lhsT=es2[:], rhs=rhs_ap(or2, D1),
                             start=False, stop=True)
            nc.scalar.activation(st0[:, oi, :, :],
                                 ps0[:].rearrange("r (b w) -> r b w", b=B),
                                 Act.Identity, bias=negthr[0:HB, p:p + 1])
            ps1_on_act = (i % 3 == 2) and g < n_lgrp - 1
            if ps1_on_act:
                nc.scalar.activation(st1[:, oi, :, :],
                                     ps1[:].rearrange("r (b w) -> r b w", b=B),
                                     Act.Identity, bias=negthr[0:HB, p:p + 1])
            else:
                nc.vector.tensor_scalar_add(
                    out=st1[:, oi, :, :],
                    in0=ps1[:].rearrange("r (b w) -> r b w", b=B),
                    scalar1=negthr[0:HB, p:p + 1])
            if oi == OGRP - 1:
                e0 = nc.sync if og % 2 == 0 else nc.scalar
                e1 = nc.scalar if og % 2 == 0 else nc.sync
                e0.dma_start(out=out0_ap[:, og * OGRP:(og + 1) * OGRP],
                             in_=st0[:])
                e1.dma_start(out=out1_ap[:, og * OGRP:(og + 1) * OGRP],
                             in_=st1[:])

    ctx.close()


_COMPILED = {}


def _get_compiled():
    if "nc" not in _COMPILED:
        nc = bacc.Bacc("TRN2", target_bir_lowering=False, debug=False,
                       num_devices=N_CORES)
        build_device_program(nc)
        nc.compile()
        _COMPILED["nc"] = nc
    return _COMPILED["nc"]


def _ensure_ntff_hook():
    """The agent image's antenv lacks axon_hooks; shim it so trace=True can
    drive NTFF profiling via the boot module's ctypes hook (test-only path)."""
    import types

    try:
        from antenv.axon_hooks import get_axon_ntff_profile_hook  # noqa: F401
        return
    except ImportError:
        pass
    import antenv

    mod = types.ModuleType("antenv.axon_hooks")
    _hook = [None]
    mod.set_axon_ntff_profile_hook = lambda h: _hook.__setitem__(0, h)
    mod.get_axon_ntff_profile_hook = lambda: _hook[0]
    sys.modules["antenv.axon_hooks"] = mod
    antenv.axon_hooks = mod
    from trn_agent_boot.trn_boot import _ntff_profile_via_ctypes

    mod.set_axon_ntff_profile_hook(
        _ntff_profile_via_ctypes("/opt/axon/libaxon_pjrt.so"))


def run(inputs: dict, trace: bool = False):
    """Run on the 8 cores. Returns (full output [B,256,H,W], exec_time_ns|None)."""
    x = np.asarray(inputs["x"], dtype=np.float32).reshape(B, H, W)
    offset_x1 = np.asarray(inputs["offset_x1"], np.float32)
    offset_x2 = np.asarray(inputs["offset_x2"], np.float32)
    offset_y1 = np.asarray(inputs["offset_y1"], np.float32)
    offset_y2 = np.asarray(inputs["offset_y2"], np.float32)
    radii = np.asarray(inputs["radii"]).astype(np.int32)
    thresholds = np.asarray(inputs["thresholds"], np.float32)

    sdt = _band_matrices()
    e2 = _shift_identity()
    nc = _get_compiled()

    # host-side index prep: sy/sx = clip(floor(off), -16, 16)+16; b, t from sy
    def sclip(off):
        return np.clip(np.floor(off), -PAD, PAD).astype(np.int64) + PAD

    sy1, sx1 = sclip(offset_y1), sclip(offset_x1)
    sy2, sx2 = sclip(offset_y2), sclip(offset_x2)
    d0 = np.clip(radii.astype(np.int64), 1, 3) - 1
    n_lgrp = P_CORE // LGRP

    def tabs(sy, sx):
        b = (sy >= 16).astype(np.int64) + (sy >= 32).astype(np.int64)
        t = sy - 16 * b
        pe = b * TFREE + d0 * 512 + sx
        return pe, t

    pe1, t1 = tabs(sy1, sx1)
    pe2, t2 = tabs(sy2, sx2)
    dve1 = 128 + t1
    dve2 = EW + 128 + t2
    tab_pe = np.stack([pe1, pe2], axis=-1).astype(np.int32)    # [P_TOTAL, 2]
    tab_dve = np.stack([dve1, dve2], axis=-1).astype(np.int32)

    in_maps = []
    for c in range(N_CORES):
        sl = slice(c * P_CORE, (c + 1) * P_CORE)
        tab = np.stack([tab_pe[sl].reshape(n_lgrp, LGRP, 2),
                        tab_dve[sl].reshape(n_lgrp, LGRP, 2)])
        in_maps.append({
            "x": x,
            "nthr": -thresholds[sl].reshape(1, P_CORE).astype(np.float32),
            "tab": tab.reshape(1, 2, n_lgrp, LGRP, 2),
            "sdt": sdt,
            "e2": e2,
        })

    if trace:
        _ensure_ntff_hook()
    res = run_bass_kernel_spmd(nc, in_maps, list(range(N_CORES)), trace=trace)
    # per-core out0/out1 [112, P_CORE, B, W] bf16 (rows 0..111 / 112..223)
    full = np.empty((B, P_TOTAL, H, W), np.float32)
    for c in range(N_CORES):
        o0 = np.asarray(res.results[c]["out0"]).astype(np.float32)
        o1 = np.asarray(res.results[c]["out1"]).astype(np.float32)
        sl = slice(c * P_CORE, (c + 1) * P_CORE)
        full[:, sl, :HB, :] = o0.transpose(2, 1, 0, 3)
        full[:, sl, HB:, :] = o1.transpose(2, 1, 0, 3)
    return full, res.exec_time_ns


def kernel(x, offset_x1, offset_x2, offset_y1, offset_y2, radii, thresholds,
           max_radius):
    out, _ = run({
        "x": x, "offset_x1": offset_x1, "offset_x2": offset_x2,
        "offset_y1": offset_y1, "offset_y2": offset_y2,
        "radii": radii, "thresholds": thresholds, "max_radius": max_radius,
    })
    return out


if __name__ == "__main__":
    # smoke test with random data
    rng = np.random.default_rng(0)
    out = kernel(
        x=rng.standard_normal((B, 1, H, W), dtype=np.float32),
        offset_x1=rng.uniform(-16, 16, P_TOTAL).astype(np.float32),
        offset_x2=rng.uniform(-16, 16, P_TOTAL).astype(np.float32),
        offset_y1=rng.uniform(-16, 16, P_TOTAL).astype(np.float32),
        offset_y2=rng.uniform(-16, 16, P_TOTAL).astype(np.float32),
        radii=rng.integers(1, 4, P_TOTAL).astype(np.int32),
        thresholds=(rng.standard_normal(P_TOTAL) * 0.1).astype(np.float32),
        max_radius=3,
    )
    print("out", out.shape, out.dtype, float(np.abs(out).max()))
